# revision 1
# baseline (speedup 1.0000x reference)
"""EnVAE sampling kernel for 8x TRN2 NeuronCores.

Math (per group g, batch element b):
  Xg = X[:, g::8]                                     # (b, 128)
  h  = relu(Xg @ W1[g] + b1[g])                        # (b, 128)
  out= h @ W2[g] + b2[g]; means=out[:, :64]; lv=out[:, 64:]
  z  = means[b, idx] + eps * exp(0.5 * lv[b, idx])

Device computes (batch-sharded 8 ways, fp16 matmuls):
  zM[g,b] = (W2m[g]^T h)[idx[g,b], b]        (via onehot Hadamard + reduce-matmul)
  zX[g,b] = exp(0.5*L + 0.5*b2v[g])[idx[g,b], b]
Host finishes: z = zM + b2m[g, idx] + eps * zX
"""

import numpy as np
import ml_dtypes

import concourse.bass as bass
import concourse.bacc as bacc
import concourse.mybir as mybir
from concourse import tile
from concourse import bass_utils

OBS = 1024
LAT = 64
G = 8
GS = 128
HID = 128
BATCH = 65536
NCORES = 8
BPC = BATCH // NCORES        # 8192 batch rows per core
SC = 512                     # batch rows per superchunk
NPAIR = G // 2
BF16 = mybir.dt.float16  # fp16: same PE rate as bf16, 8x mantissa
F32 = mybir.dt.float32

# group n takes columns n, n+8, ... (round-robin)
GROUP_IDX = np.stack([np.arange(n, OBS, G) for n in range(G)])  # (g, gs)


def build_program(nsc: int, num_devices: int = NCORES):
    """Build the per-core bass program for nsc superchunks of SC batch rows."""
    B = nsc * SC
    nc = bacc.Bacc("TRN2", target_bir_lowering=False, debug=False,
                   num_devices=num_devices)

    QUAD = 4 if nsc % 4 == 0 else 1
    nquad = nsc // QUAD
    QW = QUAD * SC
    # DRAM inputs (per-core shard)
    # xt: quad-block-major [nquad, G, QW, GS] fp16
    xt = nc.dram_tensor("xt", [nquad, G, QW, GS], BF16, kind="ExternalInput").ap()
    # onehot, transposed per pair: [nquad, NPAIR, 128, QW] int8
    #   partitions 0:64   = onehot[g=2*pair]   (latent on partition)
    #   partitions 64:128 = onehot[g=2*pair+1]
    oh = nc.dram_tensor("oh", [nquad, NPAIR, 128, QW], mybir.dt.int8,
                        kind="ExternalInput").ap()
    w1 = nc.dram_tensor("w1", [G, GS, HID], BF16, kind="ExternalInput").ap()
    # w2 packed per pair: [NPAIR, 2(tensor: m/v), GS, 2(group), LAT] bf16
    w2m = nc.dram_tensor("w2m", [G, GS, LAT], BF16, kind="ExternalInput").ap()
    w2v = nc.dram_tensor("w2v", [G, GS, LAT], BF16, kind="ExternalInput").ap()
    b1 = nc.dram_tensor("b1", [G, GS], F32, kind="ExternalInput").ap()
    # hb2v[pair] = per-partition bias col for exp: [NPAIR, 128] f32
    hb2v = nc.dram_tensor("hb2v", [NPAIR, 128], F32, kind="ExternalInput").ap()
    # selector for the reduce matmul: [2, 128, 4] bf16
    sel = nc.dram_tensor("sel", [2, 128, 4], BF16, kind="ExternalInput").ap()
    # output: [128, nsc*NPAIR*16] f32; col = ((sc*NPAIR + pair)*4 + c)*4 + q
    zout = nc.dram_tensor("z", [128, nsc * NPAIR * 16], F32,
                          kind="ExternalOutput").ap()

    from contextlib import ExitStack
    with tile.TileContext(nc) as tc, ExitStack() as st:
        # --- resident constants ---
        cp = st.enter_context(tc.tile_pool(name="const", bufs=1))
        if True:
            w1_sb = cp.tile([GS, G, HID], BF16, tag="w1")
            nc.sync.dma_start(w1_sb[:], w1.rearrange("g k m -> k g m"))
            w2m_sb = cp.tile([GS, G, LAT], BF16, tag="w2m")
            nc.sync.dma_start(w2m_sb[:], w2m.rearrange("g k m -> k g m"))
            w2v_sb = cp.tile([GS, G, LAT], BF16, tag="w2v")
            nc.sync.dma_start(w2v_sb[:], w2v.rearrange("g k m -> k g m"))
            b1_sb = cp.tile([GS, G], F32, tag="b1")
            nc.sync.dma_start(b1_sb[:], b1.rearrange("g k -> k g"))
            hb2v_sb = cp.tile([128, NPAIR], F32, tag="hb2v")
            nc.sync.dma_start(hb2v_sb[:], hb2v.rearrange("p k -> k p"))
            sel_sb = cp.tile([128, 2, 4], BF16, tag="sel")
            nc.sync.dma_start(sel_sb[:], sel.rearrange("t k f -> k t f"))

            # persistent z staging + z psum banks
            zpool = st.enter_context(tc.tile_pool(name="zp", bufs=1, space="PSUM"))
            xpool = st.enter_context(tc.tile_pool(name="xt", bufs=16))
            ohpool = st.enter_context(tc.tile_pool(name="ohp", bufs=8))
            hpsum = st.enter_context(tc.tile_pool(name="hps", bufs=3, space="PSUM"))
            hpool = st.enter_context(tc.tile_pool(name="hsb", bufs=8))
            mvpsum = st.enter_context(tc.tile_pool(name="mvps", bufs=2, space="PSUM"))
            ppool = st.enter_context(tc.tile_pool(name="prod", bufs=8))
            zsbp = st.enter_context(tc.tile_pool(name="zsb", bufs=1))

            if True:
                ZCOLS = 16  # cols per (pair, sc) in the z psum tile: 4 chunks x 4 q
                # one z psum tile per 32 (pair,sc) instances (512 cols each)
                nzt = (nsc * NPAIR + 31) // 32
                ztiles = [zpool.tile([128, 512], F32, name=f"zt{i}", tag="z")
                          for i in range(nzt)]
                zsb = zsbp.tile([128, nsc * NPAIR * 16], F32, tag="zstage")

                pending = []
                stage2 = []
                drained = set()

                def _emit_stage2(item):
                    inst, bM, bX, oht_, pr = item
                    prodM = ppool.tile([128, SC], BF16, name="prodM",
                                       tag="prodM")
                    nc.vector.tensor_tensor(prodM[:], bM[:], oht_,
                                            mybir.AluOpType.mult)
                    xsb = ppool.tile([128, SC], BF16, name="xsb", tag="xsb")
                    nc.scalar.activation(
                        xsb[:], bX[:],
                        mybir.ActivationFunctionType.Exp,
                        bias=hb2v_sb[:, pr:pr + 1], scale=0.5)
                    prodX = ppool.tile([128, SC], BF16, name="prodX",
                                       tag="prodX")
                    nc.gpsimd.tensor_tensor(prodX[:], xsb[:], oht_,
                                            mybir.AluOpType.mult)
                    pending.append((inst, prodM, prodX))

                def _drain(done_tile_idx):
                    # after the last zred of a z tile, copy it out so the
                    # single psum slot can recycle
                    if done_tile_idx is not None:
                        i = done_tile_idx
                        w = min(512, nsc * NPAIR * 16 - i * 512)
                        nc.scalar.copy(zsb[:, i * 512:i * 512 + w],
                                       ztiles[i][:, :w])
                        drained.add(i)

                def _emit_zred(item):
                    inst, pM, pX = item
                    zt = ztiles[inst // 32]
                    zoff = (inst % 32) * ZCOLS
                    for c in range(4):
                        zslice = zt[:, zoff + 4 * c: zoff + 4 * c + 4]
                        nc.tensor.matmul(
                            zslice, pM[:, 128 * c:128 * c + 128],
                            sel_sb[:, 0], start=True, stop=False,
                            skip_group_check=True)
                        nc.tensor.matmul(
                            zslice, pX[:, 128 * c:128 * c + 128],
                            sel_sb[:, 1], start=False, stop=True,
                            skip_group_check=True)

                for quad in range(nquad):
                    # --- bulk loads: XgT for all 8 groups, oh for all pairs
                    xg = [xpool.tile([GS, QW], BF16, name=f"xg{g}", tag="xg")
                          for g in range(G)]
                    for g in range(G):
                        nc.sync.dma_start(xg[g][:], xt[quad, g], transpose=True)
                    ohq = [ohpool.tile([128, QW], mybir.dt.int8,
                                        name=f"oh{p}", tag="oh")
                           for p in range(NPAIR)]
                    for p in range(NPAIR):
                        nc.sync.dma_start(ohq[p][:], oh[quad, p])

                    for scq in range(QUAD):
                        sc = quad * QUAD + scq
                        so = scq * SC
                        for pair in range(NPAIR):
                            g0, g1 = 2 * pair, 2 * pair + 1
                            oht = ohq[pair][:, so:so + SC]

                            # --- mm1 + relu per group (relu alternates ACT/DVE)
                            hsb = [hpool.tile([HID, SC], BF16, name=f"hsb{_i}",
                                              tag="h") for _i in range(2)]
                            for i, g in enumerate((g0, g1)):
                                hp = hpsum.tile([HID, SC], F32, tag="hpsum")
                                nc.tensor.matmul(hp[:], w1_sb[:, g],
                                                 xg[g][:, so:so + SC],
                                                 start=True, stop=True)
                                if i == 0:
                                    # g0 relu on ACT, g1 on DVE: they run
                                    # concurrently, unblocking mm2 sooner
                                    nc.scalar.activation(
                                        hsb[i][:], hp[:],
                                        mybir.ActivationFunctionType.Relu,
                                        bias=b1_sb[:, g:g + 1], scale=1.0)
                                else:
                                    nc.vector.tensor_scalar(
                                        hsb[i][:], hp[:],
                                        b1_sb[:, g:g + 1], 0.0,
                                        mybir.AluOpType.add,
                                        mybir.AluOpType.max)

                            # --- mm2: col-packed pairs (means first) ---
                            bankM = mvpsum.tile([128, SC], F32, tag="bankM")
                            bankX = mvpsum.tile([128, SC], F32, tag="bankX")
                            for i, g in enumerate((g0, g1)):
                                nc.tensor.matmul(bankM[64 * i:64 * i + 64, :],
                                                 w2m_sb[:, g], hsb[i][:],
                                                 start=True, stop=True,
                                                 tile_position=(0, 64 * i))
                            for i, g in enumerate((g0, g1)):
                                nc.tensor.matmul(bankX[64 * i:64 * i + 64, :],
                                                 w2v_sb[:, g], hsb[i][:],
                                                 start=True, stop=True,
                                                 tile_position=(0, 64 * i))

                            # --- stage-2 (Hadamard + exp) for the PREVIOUS
                            # iteration: keeps every engine FIFO free of
                            # head-of-line waits on just-issued matmuls
                            inst = sc * NPAIR + pair
                            stage2.append((inst, bankM, bankX, oht, pair))
                            if len(stage2) > 1:
                                _emit_stage2(stage2.pop(0))
                            if len(pending) > 2:
                                _drain(_emit_zred(pending.pop(0)))

                for item in stage2:
                    _emit_stage2(item)
                for item in pending:
                    _drain(_emit_zred(item))
                for i, zt in enumerate(ztiles):
                    if i not in drained:
                        w = min(512, nsc * NPAIR * 16 - i * 512)
                        nc.vector.tensor_copy(zsb[:, i * 512:i * 512 + w],
                                              zt[:, :w])
                nc.sync.dma_start(zout[:], zsb[:])

    nc.compile()
    return nc


# ---------------------------------------------------------------- host side --

def _prep_host(X, eps, W1, b1, W2, b2, indices, nsc=BPC // SC, ncores=NCORES):
    """Build per-core input dicts + closures for unscrambling."""
    B = nsc * SC
    bf = np.float16
    # X: permute columns group-major, cast bf16, block layout [nsc, G, SC, GS]
    Xp = np.ascontiguousarray(X[:, GROUP_IDX.reshape(-1)]).astype(bf)  # (BATCH, 1024)
    W1b = W1.astype(bf)                              # (g, gs, hid)
    W2m = np.ascontiguousarray(W2[:, :, :LAT]).astype(bf)
    W2v = np.ascontiguousarray(W2[:, :, LAT:]).astype(bf)
    b1f = b1.astype(np.float32)
    hb2v = np.zeros((NPAIR, 128), np.float32)
    for p in range(NPAIR):
        hb2v[p, :64] = 0.5 * b2[2 * p, LAT:]
        hb2v[p, 64:] = 0.5 * b2[2 * p + 1, LAT:]
    selm = np.zeros((2, 128, 4), np.float32)
    selm[0, :64, 0] = 1.0   # zM g0
    selm[0, 64:, 1] = 1.0   # zM g1
    selm[1, :64, 2] = 1.0   # zX g0
    selm[1, 64:, 3] = 1.0   # zX g1
    selb = selm.astype(bf)

    QUAD = 4 if nsc % 4 == 0 else 1
    nquad = nsc // QUAD
    QW = QUAD * SC
    in_maps = []
    for core in range(ncores):
        lo = core * B
        Xc = Xp[lo:lo + B].reshape(nquad, QW, G, GS)
        xt = np.ascontiguousarray(Xc.transpose(0, 2, 1, 3))      # (nq,G,QW,GS)
        idxc = indices[:, lo:lo + B]                             # (G, B)
        ohc = np.zeros((nquad, NPAIR, 128, QW), np.float32)
        ar = np.arange(LAT)
        for p in range(NPAIR):
            for i, g in enumerate((2 * p, 2 * p + 1)):
                ii = idxc[g].reshape(nquad, QW)                  # (nq, QW)
                m = (ii[:, None, :] == ar[None, :, None])        # (nq, 64, QW)
                ohc[:, p, 64 * i:64 * i + 64, :] = m
        in_maps.append({
            "xt": xt, "oh": ohc.astype(np.int8), "w1": W1b, "w2m": W2m, "w2v": W2v,
            "b1": b1f, "hb2v": hb2v, "sel": selb,
        })
    return in_maps


def _unscramble(zdev, nsc):
    """zdev: (128, nsc*NPAIR*16) f32 -> zM, zX each (G, nsc*SC)."""
    B = nsc * SC
    zr = zdev.reshape(128, nsc, NPAIR, 4, 4)       # p, sc, pair, c, q
    zM = np.zeros((G, B), np.float32)
    zX = np.zeros((G, B), np.float32)
    for pair in range(NPAIR):
        for q, (dst, g) in enumerate(((zM, 2 * pair), (zM, 2 * pair + 1),
                                      (zX, 2 * pair), (zX, 2 * pair + 1))):
            blk = zr[:, :, pair, :, q]             # (128, nsc, 4)
            dst[g] = blk.transpose(1, 2, 0).reshape(B)
    return zM, zX


_NC_CACHE = {}


def kernel(X, eps, W1, b1, W2, b2, indices):
    nsc = BPC // SC
    key = (nsc, NCORES)
    if key not in _NC_CACHE:
        _NC_CACHE[key] = build_program(nsc, NCORES)
    nc = _NC_CACHE[key]
    in_maps = _prep_host(X, eps, W1, b1, W2, b2, indices)
    res = bass_utils.run_bass_kernel_spmd(nc, in_maps, core_ids=list(range(NCORES)))

    z = np.zeros((G, BATCH), np.float32)
    B = nsc * SC
    for core in range(NCORES):
        lo = core * B
        zM, zX = _unscramble(res.results[core]["z"], nsc)
        idxc = indices[:, lo:lo + B]
        b2m_sel = np.take_along_axis(b2[:, :LAT], idxc, axis=1)
        z[:, lo:lo + B] = zM + b2m_sel + eps[:, lo:lo + B] * zX
    return z.astype(np.float32)



# revision 4
# speedup vs baseline: 1.9342x; 1.9342x over previous
"""EnVAE sampling kernel for 8x TRN2 NeuronCores — sorted-selection design.

Math (per group g, batch element b):
  Xg = X[:, g::8]                                      # (b, 128)
  h  = relu(Xg @ W1[g] + b1[g])                        # (b, 128)
  out= h @ W2[g] + b2[g]; means=out[:, :64]; lv=out[:, 64:]
  z  = means[b, idx] + eps * exp(0.5 * lv[b, idx])

Device strategy (per core):
  Host sorts each group's batch by latent index and balances counts across
  cores, so each (group, latent) block is exactly C=128 columns (underfull
  blocks padded with dummies, overfull spill to host numpy).
  - mm1: fp8 DoubleRow matmul  W1dr[64,2,128] x Xdr[64,2,256] -> h PSUM
  - relu+bias: PSUM->SBUF fp16, rotated across ACT/DVE/Pool engines
  - select:   per latent l, matmul(out[128,2], lhsT=h[:,128l:128l+128],
              rhs=W2mv[g,l][128,2]) -> z block in PSUM (batch on partitions)
  - one staging copy + one DMA out.
Host finishes: z = zM + b2m[g,idx] + eps * exp(0.5*zLV + 0.5*b2v[g,idx]).
"""

import numpy as np
import ml_dtypes

import concourse.bass as bass
import concourse.bacc as bacc
import concourse.mybir as mybir
from concourse import tile
from concourse import bass_utils

OBS = 1024
LAT = 64
G = 8
GS = 128
HID = 128
BATCH = 65536
NCORES = 8
BPC = BATCH // NCORES        # 8192 batch rows per core
C = 128                      # columns per (group, latent) block
SC = 512                     # kept for test.py compat (unused)
NPAIR = G // 2               # kept for test.py compat (unused)
F8 = mybir.dt.float8e4
F16 = mybir.dt.float16
F32 = mybir.dt.float32
NPF8 = ml_dtypes.float8_e4m3

# group n takes columns n, n+8, ... (round-robin)
GROUP_IDX = np.stack([np.arange(n, OBS, G) for n in range(G)])  # (g, gs)

# relu engine rotation per 1024-col chunk (8 chunks/group). GPSIMD cannot
# read PSUM, so only ACT (~996ns) and DVE (~1190ns) can drain mm1 output;
# weight the rotation by their throughputs (ACT 9/16, DVE 7/16).
RELU_ROT = ["act", "dve", "act", "dve", "act", "dve", "act", "act",
            "dve", "act", "dve", "act", "dve", "act", "dve", "act"]


def build_program(nsc: int = None, num_devices: int = NCORES):
    """Per-core bass program. Data-independent (fixed block size C)."""
    nc = bacc.Bacc("TRN2", target_bir_lowering=False, debug=False,
                   num_devices=num_devices)

    NB = LAT * C                  # 8192 padded batch cols per group
    NCH = NB // 1024              # 8 relu chunks per group

    xt = nc.dram_tensor("xt", [G, 64, 2, NB], F8, kind="ExternalInput").ap()
    w1 = nc.dram_tensor("w1", [64, G, 2, HID], F8, kind="ExternalInput").ap()
    b1 = nc.dram_tensor("b1", [GS, G], F32, kind="ExternalInput").ap()
    w2 = nc.dram_tensor("w2", [HID, G * 2 * LAT], F16, kind="ExternalInput").ap()
    zout = nc.dram_tensor("z", [C, G * 2 * LAT], F16, kind="ExternalOutput").ap()

    from contextlib import ExitStack
    with tile.TileContext(nc) as tc, ExitStack() as st:
        cp = st.enter_context(tc.tile_pool(name="const", bufs=1))
        w1_sb = cp.tile([64, G, 2, HID], F8, tag="w1")
        nc.sync.dma_start(w1_sb[:], w1)
        w2_sb = cp.tile([HID, G * 2 * LAT], F16, tag="w2")
        nc.sync.dma_start(w2_sb[:], w2)
        b1_sb = cp.tile([GS, G], F32, tag="b1")
        nc.sync.dma_start(b1_sb[:], b1)

        xpool = st.enter_context(tc.tile_pool(name="xp", bufs=2))
        hpool = st.enter_context(tc.tile_pool(name="hp", bufs=2))
        hpsum = st.enter_context(tc.tile_pool(name="hps", bufs=3, space="PSUM"))
        zpsum = st.enter_context(tc.tile_pool(name="zps", bufs=1, space="PSUM"))
        zsbp = st.enter_context(tc.tile_pool(name="zsb", bufs=1))

        ztiles = [zpsum.tile([C, 512], F32, name=f"zt{i}", tag="z")
                  for i in range(2)]
        zsb = zsbp.tile([C, G * 2 * LAT], F16, tag="zstage")

        for g in range(G):
            xg = xpool.tile([64, 2, NB], F8, name=f"xg{g}", tag="xg")
            nc.sync.dma_start(xg[:], xt[g])
            hg = hpool.tile([HID, NB], F16, name=f"h{g}", tag="h")

            for c in range(NCH):
                hp = hpsum.tile([HID, 1024], F32, tag="hpsum")
                for q in range(4):
                    col = c * 1024 + q * 256
                    nc.tensor.matmul(
                        hp[:, q * 256:(q + 1) * 256], w1_sb[:, g],
                        xg[:, :, col:col + 256],
                        start=True, stop=True,
                        perf_mode=mybir.MatmulPerfMode.DoubleRow)
                dst = hg[:, c * 1024:(c + 1) * 1024]
                eng = RELU_ROT[(g * NCH + c) % len(RELU_ROT)]
                if eng == "act":
                    nc.scalar.activation(
                        dst, hp[:], mybir.ActivationFunctionType.Relu,
                        bias=b1_sb[:, g:g + 1], scale=1.0)
                else:
                    nc.vector.tensor_scalar(
                        dst, hp[:], b1_sb[:, g:g + 1], 0.0,
                        mybir.AluOpType.add, mybir.AluOpType.max)

            zt = ztiles[g // 4]
            base = (g % 4) * 2 * LAT
            for l in range(LAT):
                nc.tensor.matmul(
                    zt[:, base + 2 * l: base + 2 * l + 2],
                    hg[:, l * C:(l + 1) * C],
                    w2_sb[:, g * 2 * LAT + 2 * l: g * 2 * LAT + 2 * l + 2],
                    start=True, stop=True, skip_group_check=True)

            if g == 3:
                nc.scalar.copy(zsb[:, :512], ztiles[0][:])
            elif g == 7:
                nc.vector.tensor_copy(zsb[:, 512:], ztiles[1][:])
        nc.sync.dma_start(zout[:], zsb[:])

    nc.compile()
    return nc


# ---------------------------------------------------------------- host side --

def _plan(indices):
    """Sort/balance each group's batch into (core, latent, slot) blocks.

    Returns colmap [ncores, G, LAT*C] int32 (batch idx per padded column,
    -1 for dummy pad) and spill mask [G, BATCH] (elements computed on host).
    """
    colmap = np.full((NCORES, G, LAT * C), -1, np.int64)
    spill = np.zeros((G, BATCH), bool)
    for g in range(G):
        idxg = indices[g].astype(np.int64)
        order = np.argsort(idxg, kind="stable")          # batch sorted by latent
        counts = np.bincount(idxg, minlength=LAT)
        starts = np.concatenate([[0], np.cumsum(counts)[:-1]])
        r = np.arange(BATCH) - np.repeat(starts, counts)  # rank within latent
        core = r % NCORES
        slot = r // NCORES
        lat = idxg[order]
        ok = slot < C
        spill[g, order[~ok]] = True
        pos = lat * C + slot
        for k in range(NCORES):
            m = ok & (core == k)
            colmap[k, g, pos[m]] = order[m]
    return colmap, spill


def _prep_host(X, eps, W1, b1, W2, b2, indices, **_):
    """Build per-core input dicts. Returns (in_maps, colmap, spill)."""
    colmap, spill = _plan(indices)
    # group-major X in fp8: Xp8[b, g*128+f] = fp8(X[b, GROUP_IDX[g][f]])
    Xp8 = np.ascontiguousarray(X[:, GROUP_IDX.reshape(-1)]).astype(NPF8)
    w1dr = np.ascontiguousarray(
        W1.astype(NPF8).reshape(G, 2, 64, HID).transpose(2, 0, 1, 3))  # (64,G,2,H)
    b1f = np.ascontiguousarray(b1.astype(np.float32).T)                # (128,G)
    # w2 moving operand: col (g, l, j): j=0 -> W2[g][:, l], j=1 -> W2[g][:, 64+l]
    w2m = W2[:, :, :LAT]
    w2v = W2[:, :, LAT:]
    w2sel = np.stack([w2m, w2v], axis=-1)            # (G, H, LAT, 2)
    w2sel = np.ascontiguousarray(
        w2sel.transpose(1, 0, 2, 3).reshape(HID, G * LAT * 2)).astype(np.float16)

    in_maps = []
    for k in range(NCORES):
        xt = np.empty((G, 64, 2, LAT * C), NPF8)
        for g in range(G):
            cm = colmap[k, g]
            rows = np.where(cm < 0, 0, cm)
            xg = Xp8[rows, g * GS:(g + 1) * GS]      # (8192, 128) fp8
            xt[g] = xg.T.reshape(2, 64, LAT * C).transpose(1, 0, 2)
        in_maps.append({"xt": xt, "w1": w1dr, "b1": b1f, "w2": w2sel})
    return in_maps, colmap, spill


def _finish(results, inputs, colmap, spill):
    """Combine device outputs + host-side math into z (G, BATCH) f32."""
    X, eps, W1, b1, W2, b2, indices = (
        inputs["X"], inputs["eps"], inputs["W1"], inputs["b1"],
        inputs["W2"], inputs["b2"], inputs["indices"])
    zM = np.zeros((G, BATCH), np.float32)
    zLV = np.zeros((G, BATCH), np.float32)
    for k in range(NCORES):
        zdev = np.asarray(results[k]["z"], np.float32)   # (128, G*128)
        for g in range(G):
            zg = zdev[:, g * 2 * LAT:(g + 1) * 2 * LAT]  # (slot 128, 2*LAT)
            # column 2l+j -> (lat l, j); want per pos = l*C + slot
            zper = zg.reshape(C, LAT, 2).transpose(1, 0, 2).reshape(LAT * C, 2)
            cm = colmap[k, g]
            ok = cm >= 0
            zM[g, cm[ok]] = zper[ok, 0]
            zLV[g, cm[ok]] = zper[ok, 1]

    # host-side spilled elements (exact f32 math)
    for g in range(G):
        bs = np.where(spill[g])[0]
        if len(bs) == 0:
            continue
        Xg = X[bs][:, GROUP_IDX[g]].astype(NPF8).astype(np.float32)
        h = np.maximum(
            Xg @ W1[g].astype(NPF8).astype(np.float32) + b1[g], 0.0)
        idxs = indices[g, bs]
        w2mc = W2[g][:, idxs]            # (H, n)
        w2vc = W2[g][:, LAT + idxs]
        zM[g, bs] = np.einsum("nh,hn->n", h, w2mc)
        zLV[g, bs] = np.einsum("nh,hn->n", h, w2vc)

    b2m_sel = np.take_along_axis(b2[:, :LAT], indices, axis=1)
    b2v_sel = np.take_along_axis(b2[:, LAT:], indices, axis=1)
    z = zM + b2m_sel + eps * np.exp(0.5 * zLV + 0.5 * b2v_sel)
    return z.astype(np.float32)


_NC_CACHE = {}


def kernel(X, eps, W1, b1, W2, b2, indices):
    key = NCORES
    if key not in _NC_CACHE:
        _NC_CACHE[key] = build_program(num_devices=NCORES)
    nc = _NC_CACHE[key]
    inputs = {"X": X, "eps": eps, "W1": W1, "b1": b1, "W2": W2, "b2": b2,
              "indices": indices}
    in_maps, colmap, spill = _prep_host(**inputs)
    res = bass_utils.run_bass_kernel_spmd(nc, in_maps,
                                          core_ids=list(range(NCORES)))
    return _finish(res.results, inputs, colmap, spill)
